# revision 55
# baseline (speedup 1.0000x reference)
"""Multi-head attention (ViT-style, B=32 N=577 C=768 H=12) on 8 TRN2 NeuronCores.

Sharding: pure data-parallel over batch — each core gets 4 batches plus a
replicated copy of the (host-preprocessed) weights. No collectives.

Per-core pipeline (all matmuls bf16 with fp32 PSUM accumulation):
  stage 1a: QK^T = [Wq*scale; Wk]^T-major matmul  -> qk  [1536, 577] (o-major)
  stage 1b: V'   = x @ Wv^T laid out per-head with a ones column  (for softmax sums)
  stage 2 per head: S^T = K^T.T@Q^T -> exp (no max-sub; scores are O(1)) ->
            O'^T = V'.T @ expS^T  (row 64 = softmax denominators) ->
            reciprocal + partition_broadcast + multiply -> C^T
  stage 3: out = C @ Wp^T + b  (C^T tiles feed matmul lhsT directly)
"""

import sys

sys.path.insert(0, "/opt/trn_rl_repo")

import ml_dtypes
import numpy as np

import concourse.bass as bass  # noqa: F401  (registers AP machinery)
import concourse.mybir as mybir
import concourse.tile as tile
from concourse import bacc, bass_utils

DIM = 768
H = 12
D = 64
N = 577
B = 32
NCORES = 8
BLOC = B // NCORES
SCALE = D**-0.5

BF16 = mybir.dt.bfloat16
F32 = mybir.dt.float32

# token/key chunks along a 577 axis mapped to <=128 partitions
PCH = [(0, 128), (128, 128), (256, 128), (384, 128), (512, 65)]
# free-dim chunks along a 577 axis (<=512 per PSUM bank)
NCH = [(0, 512), (512, 65)]
# free-dim chunks along the 768 output-feature axis
OCH = [(0, 512), (512, 256)]

_NC_CACHE = None


def _build(tc, xT, wqkT, wvT, wpT, bias, out):
    nc = tc.nc
    exp = mybir.ActivationFunctionType.Exp

    with (
        tc.tile_pool(name="w", bufs=1) as wpool,
        tc.tile_pool(name="xb", bufs=2) as xpool,
        tc.tile_pool(name="qk", bufs=2) as qkpool,
        tc.tile_pool(name="vp", bufs=2) as vppool,
        tc.tile_pool(name="es", bufs=6) as espool,
        tc.tile_pool(name="ct", bufs=2) as ctpool,
        tc.tile_pool(name="sm", bufs=6) as smpool,
        tc.tile_pool(name="ob", bufs=3) as opool,
        tc.tile_pool(name="ps", bufs=6, space="PSUM") as pspool,
        tc.tile_pool(name="pso", bufs=2, space="PSUM") as psopool,
    ):
        # Split the big weight loads per contraction-chunk so the first
        # matmuls can start as soon as chunk 0 lands.
        wqk_sb = wpool.tile([128, 6, 2 * DIM], BF16)
        wqk_dr = wqkT.ap().rearrange("(j p) o -> p j o", p=128)
        wv_sb = wpool.tile([128, 6, DIM], BF16)
        wv_dr = wvT.ap().rearrange("(j p) o -> p j o", p=128)
        wp_sb = wpool.tile([128, 6, DIM], BF16)
        wp_dr = wpT.ap().rearrange("(j p) o -> p j o", p=128)
        # Per-batch state, filled lazily by the emit helpers below.
        st = [dict() for _ in range(BLOC)]

        def load_x(b):
            xt = xpool.tile([128, 6, N], BF16, tag="xt")
            xt_dr = xT.ap()[b].rearrange("(j p) t -> p j t", p=128)
            for c in range(6):
                nc.sync.dma_start(xt[:, c], xt_dr[:, c])
            st[b]["xt"] = xt

        # First matmul needs xt(0) chunk 0 + wqk chunk 0 — issue those first;
        # wp/bias (stage-3 only) go last.
        xt0 = xpool.tile([128, 6, N], BF16, tag="xt")
        xt0_dr = xT.ap()[0].rearrange("(j p) t -> p j t", p=128)
        st[0]["xt"] = xt0
        for c in range(6):
            nc.sync.dma_start(xt0[:, c], xt0_dr[:, c])
            nc.sync.dma_start(wqk_sb[:, c], wqk_dr[:, c])
        for c in range(6):
            nc.sync.dma_start(wv_sb[:, c], wv_dr[:, c])
        for c in range(6):
            nc.sync.dma_start(wp_sb[:, c], wp_dr[:, c])
        bias_sb = wpool.tile([128, DIM], F32)
        nc.sync.dma_start(bias_sb[:], bias.ap())

        # Warm-up burst: the PE idles while the first DMAs land; dummy
        # matmuls there flip the HAM clock gate to 8/8 before real work.
        warm = wpool.tile([128, 512], BF16, name="warm")
        nc.gpsimd.memset(warm[:], 0.0)
        warm_ps = pspool.tile([128, 512], F32, tag="mm", name="warm_ps")
        for _ in range(18):
            nc.tensor.matmul(
                warm_ps[:, :], warm[:, 0:128], warm[:, 0:512], start=True, stop=True
            )

        # stage-1 work units: 12 QK o-tile groups (ordered so head-pair i's
        # Q and K tiles land early) + 10 V' groups = 22 units per batch.
        QK_ORDER = [0, 6, 1, 7, 2, 8, 3, 9, 4, 10, 5, 11]

        def stage1_unit(b, u):
            xt = st[b]["xt"]
            if u < 12:
                # QK^T [1536, 577]; o-tile j covers rows 128j..128j+127
                j = QK_ORDER[u]
                if "qk" not in st[b]:
                    st[b]["qk"] = qkpool.tile([128, 12, N], BF16, tag="qk", name="qk")
                qk = st[b]["qk"]
                for n0, nsz in NCH:
                    ps = pspool.tile([128, 512], F32, tag="mm")
                    for c in range(6):
                        nc.tensor.matmul(
                            ps[:, :nsz],
                            wqk_sb[:, c, j * 128 : (j + 1) * 128],
                            xt[:, c, n0 : n0 + nsz],
                            start=(c == 0),
                            stop=(c == 5),
                        )
                    nc.vector.tensor_copy(qk[:, j, n0 : n0 + nsz], ps[:, :nsz])
            else:
                # V' [577, 12*65]: per head 64 value cols + a ones col
                it, io = divmod(u - 12, 2)
                if "vp" not in st[b]:
                    vp = vppool.tile([128, 5, H * 65], BF16, tag="vp")
                    st[b]["vp"] = vp
                    for k in range(5):
                        ones = vp[:, k].rearrange("p (h c) -> p h c", c=65)[:, :, 64:65]
                        nc.gpsimd.memset(ones, 1.0)
                vp = st[b]["vp"]
                t0, tsz = PCH[it]
                o0, osz = OCH[io]
                ps = pspool.tile([128, 512], F32, tag="mm")
                for c in range(6):
                    nc.tensor.matmul(
                        ps[:tsz, :osz],
                        xt[:, c, t0 : t0 + tsz],
                        wv_sb[:, c, o0 : o0 + osz],
                        start=(c == 0),
                        stop=(c == 5),
                    )
                nh = osz // D
                h0 = o0 // D
                src = ps[:tsz, :osz].rearrange("p (h d) -> p h d", d=D)
                dst = vp[:tsz, it].rearrange("p (h c) -> p h c", c=65)[
                    :, h0 : h0 + nh, 0:D
                ]
                nc.vector.tensor_copy(dst, src)

        def scores_pair(b, i, fillers=None):
            # Heads 2i (partitions 0:64) and 2i+1 (64:128) sit in disjoint
            # PE row-groups; adjacent K=64 matmuls run concurrently in the
            # array (tile_position auto-derives from the partition bases).
            # Between m-chunk pairs, pop a filler thunk (stage-1/3 matmuls of
            # neighboring batches) so the PE has independent work while ACT
            # drains the exps that gate the next psum slot.
            qk = st[b]["qk"]
            esA = espool.tile([128, 5, N], BF16, tag="es")
            esB = espool.tile([128, 5, N], BF16, tag="es")
            st[b].setdefault("es", {})[2 * i] = esA
            st[b]["es"][2 * i + 1] = esB
            kA, qA = qk[0:64, 6 + i], qk[0:64, i]
            kB, qB = qk[64:128, 6 + i], qk[64:128, i]
            for mc, (m0, msz) in enumerate(PCH):
                psA = pspool.tile([128, 512], F32, tag="mm", name="psA")
                psB = pspool.tile([128, 512], F32, tag="mm", name="psB")
                nc.tensor.matmul(
                    psA[:msz, :512], kA[:, m0 : m0 + msz], qA[:, 0:512],
                    start=True, stop=True,
                )
                nc.tensor.matmul(
                    psB[:msz, :512], kB[:, m0 : m0 + msz], qB[:, 0:512],
                    start=True, stop=True,
                )
                nc.scalar.activation(esA[:msz, mc, 0:512], psA[:msz, :512], exp)
                nc.scalar.activation(esB[:msz, mc, 0:512], psB[:msz, :512], exp)
                if fillers:
                    fillers.popleft()()
            # the 65-wide query tail: all 5 key-chunks share one PSUM bank
            pstA_t = pspool.tile([128, 512], F32, tag="mm", name="pstA")
            pstB_t = pspool.tile([128, 512], F32, tag="mm", name="pstB")
            pstA = pstA_t[:, 0:325].rearrange("p (m c) -> p m c", c=65)
            pstB = pstB_t[:, 0:325].rearrange("p (m c) -> p m c", c=65)
            for mc, (m0, msz) in enumerate(PCH):
                nc.tensor.matmul(
                    pstA[:msz, mc, :], kA[:, m0 : m0 + msz], qA[:, 512:577],
                    start=True, stop=True,
                )
                nc.tensor.matmul(
                    pstB[:msz, mc, :], kB[:, m0 : m0 + msz], qB[:, 512:577],
                    start=True, stop=True,
                )
            nc.scalar.activation(esA[:, :, 512:577], pstA[:, :, :], exp)
            nc.scalar.activation(esB[:, :, 512:577], pstB[:, :, :], exp)

        def pv_norm(b, h):
            jq = h // 2
            pq = (h % 2) * 64
            vp = st[b]["vp"]
            es = st[b]["es"].pop(h)
            if "ct" not in st[b]:
                st[b]["ct"] = ctpool.tile([128, 6, N], BF16, tag="ct", name="ct")
            ct = st[b]["ct"]
            # O'^T = V'.T @ expS^T ; row 64 = per-query softmax denominator.
            # The two query chunks have independent denominators, so each
            # chunk's normalize chain starts as soon as its PV group lands.
            for n0, nsz in NCH:
                pso = psopool.tile([65, 512], F32, tag="pv")
                for mc, (m0, msz) in enumerate(PCH):
                    nc.tensor.matmul(
                        pso[:65, :nsz],
                        vp[:msz, mc, h * 65 : (h + 1) * 65],
                        es[:msz, mc, n0 : n0 + nsz],
                        start=(mc == 0),
                        stop=(mc == 4),
                    )
                # custom-DVE recip mis-reads PSUM; bounce the sums row to SBUF
                sums = smpool.tile([1, 512], F32, tag="sums")
                nc.vector.tensor_copy(sums[0:1, :nsz], pso[64:65, :nsz])
                recip = smpool.tile([1, 512], F32, tag="recip")
                nc.vector.reciprocal_approx_fast(recip[0:1, :nsz], sums[0:1, :nsz])
                rb = smpool.tile([64, 512], F32, tag="rb")
                nc.gpsimd.partition_broadcast(rb[0:64, :nsz], recip[0:1, :nsz])
                nc.vector.tensor_mul(
                    ct[pq : pq + 64, jq, n0 : n0 + nsz],
                    pso[0:64, :nsz],
                    rb[0:64, :nsz],
                )

        def stage3_half(b, it, io):
            # out = C @ Wp^T + b for one (token chunk, feature chunk)
            ct = st[b]["ct"]
            t0, tsz = PCH[it]
            o0, osz = OCH[io]
            if io == 0:
                st[b].setdefault("ob", {})[it] = opool.tile(
                    [128, DIM], F32, tag="ob", name="ob"
                )
            ob = st[b]["ob"][it]
            ps = pspool.tile([128, 512], F32, tag="mm")
            for c in range(6):
                nc.tensor.matmul(
                    ps[:tsz, :osz],
                    ct[:, c, t0 : t0 + tsz],
                    wp_sb[:, c, o0 : o0 + osz],
                    start=(c == 0),
                    stop=(c == 5),
                )
            nc.vector.tensor_add(
                ob[:tsz, o0 : o0 + osz],
                ps[:tsz, :osz],
                bias_sb[:tsz, o0 : o0 + osz],
            )
            if io == 1:
                nc.sync.dma_start(out.ap()[b, t0 : t0 + tsz, :], ob[:tsz, :])

        def stage3_unit(b, it):
            stage3_half(b, it, 0)
            stage3_half(b, it, 1)

        # ---- cross-batch software pipeline.  During stage-2 of batch b
        # (where the PE repeatedly waits on ACT exps), interleave the
        # dependency-free stage-1 matmuls of batch b+1 and the stage-3
        # matmuls of batch b-1 as PE filler work.
        from collections import deque

        # Stage-1 of batch b splits into a prologue (first head-pair's QK
        # tiles + all of V') emitted during stage-2 of b-1, and "self" QK
        # units emitted inside b's own stage-2 one pair ahead of use — so
        # even the last batch's stage-2 has PE filler work.
        PRO = [0, 1] + list(range(12, 22))
        SELF = list(range(2, 12))

        for u in PRO:
            stage1_unit(0, u)
        load_x(1)
        for b in range(BLOC):
            last = b == BLOC - 1
            npairs = H // 2 - 1 if last else H // 2
            for i in range(npairs):
                fillers = deque()
                if i < 5:
                    for u in (SELF[2 * i], SELF[2 * i + 1]):
                        fillers.append(lambda b=b, u=u: stage1_unit(b, u))
                if b + 1 < BLOC:
                    for u in PRO[2 * i : 2 * i + 2]:
                        fillers.append(lambda b=b, u=u: stage1_unit(b + 1, u))
                if b >= 1 and i >= 1:
                    for k in (2 * (i - 1), 2 * (i - 1) + 1):
                        it, io = divmod(k, 2)
                        fillers.append(
                            lambda b=b, it=it, io=io: stage3_half(b - 1, it, io)
                        )
                scores_pair(b, i, fillers)
                if i >= 1:
                    pv_norm(b, 2 * i - 2)
                while fillers:
                    fillers.popleft()()
                if i >= 1:
                    pv_norm(b, 2 * i - 1)
            if last:
                # emit the final pair's scores early so ACT's last exps
                # start a slot sooner; its stage-3 fillers ride along
                fillers = deque()
                for k in (8, 9):
                    it, io = divmod(k, 2)
                    fillers.append(
                        lambda b=b, it=it, io=io: stage3_half(b - 1, it, io)
                    )
                scores_pair(b, 5, fillers)
                pv_norm(b, 8)
                while fillers:
                    fillers.popleft()()
                pv_norm(b, 9)
            if b + 2 < BLOC:
                load_x(b + 2)
            pv_norm(b, H - 2)
            pv_norm(b, H - 1)
        for it in range(5):
            stage3_unit(BLOC - 1, it)


def _build_nc():
    global _NC_CACHE
    if _NC_CACHE is not None:
        return _NC_CACHE
    nc = bacc.Bacc("TRN2", target_bir_lowering=False, debug=False)
    xT = nc.dram_tensor("xT", [BLOC, DIM, N], BF16, kind="ExternalInput")
    wqkT = nc.dram_tensor("wqkT", [DIM, 2 * DIM], BF16, kind="ExternalInput")
    wvT = nc.dram_tensor("wvT", [DIM, DIM], BF16, kind="ExternalInput")
    wpT = nc.dram_tensor("wpT", [DIM, DIM], BF16, kind="ExternalInput")
    bias = nc.dram_tensor("bias", [128, DIM], F32, kind="ExternalInput")
    out = nc.dram_tensor("out", [BLOC, N, DIM], F32, kind="ExternalOutput")
    with tile.TileContext(nc) as tc:
        _build(tc, xT, wqkT, wvT, wpT, bias, out)
    nc.compile()
    _NC_CACHE = nc
    return nc


def _prep_inputs(x, W_qkv, W_proj, b_proj):
    bf = ml_dtypes.bfloat16
    x = np.asarray(x, dtype=np.float32)
    W_qkv = np.asarray(W_qkv, dtype=np.float32)
    W_proj = np.asarray(W_proj, dtype=np.float32)
    b_proj = np.asarray(b_proj, dtype=np.float32)

    wq = W_qkv[:DIM] * np.float32(SCALE)
    wk = W_qkv[DIM : 2 * DIM]
    wv = W_qkv[2 * DIM :]
    wqkT = np.ascontiguousarray(np.concatenate([wq, wk], axis=0).T).astype(bf)
    wvT = np.ascontiguousarray(wv.T).astype(bf)
    wpT = np.ascontiguousarray(W_proj.T).astype(bf)
    bias_bc = np.ascontiguousarray(np.broadcast_to(b_proj, (128, DIM))).astype(
        np.float32
    )

    in_maps = []
    for c in range(NCORES):
        xb = x[c * BLOC : (c + 1) * BLOC]  # [BLOC, N, DIM]
        xTc = np.ascontiguousarray(xb.transpose(0, 2, 1)).astype(bf)
        in_maps.append(
            {"xT": xTc, "wqkT": wqkT, "wvT": wvT, "wpT": wpT, "bias": bias_bc}
        )
    return in_maps


def _run(x, W_qkv, W_proj, b_proj, trace=False):
    nc = _build_nc()
    in_maps = _prep_inputs(x, W_qkv, W_proj, b_proj)
    res = bass_utils.run_bass_kernel_spmd(
        nc, in_maps, core_ids=list(range(NCORES)), trace=trace
    )
    out = np.concatenate(
        [np.asarray(res.results[c]["out"], dtype=np.float32) for c in range(NCORES)],
        axis=0,
    )
    return out, res


def kernel(x, W_qkv, W_proj, b_proj):
    out, _ = _run(x, W_qkv, W_proj, b_proj, trace=False)
    return out


# revision 56
# speedup vs baseline: 1.0169x; 1.0169x over previous
"""Multi-head attention (ViT-style, B=32 N=577 C=768 H=12) on 8 TRN2 NeuronCores.

Sharding: pure data-parallel over batch — each core gets 4 batches plus a
replicated copy of the (host-preprocessed) weights. No collectives.

Per-core pipeline (all matmuls bf16 with fp32 PSUM accumulation):
  stage 1a: QK^T = [Wq*scale; Wk]^T-major matmul  -> qk  [1536, 577] (o-major)
  stage 1b: V'   = x @ Wv^T laid out per-head with a ones column  (for softmax sums)
  stage 2 per head: S^T = K^T.T@Q^T -> exp (no max-sub; scores are O(1)) ->
            O'^T = V'.T @ expS^T  (row 64 = softmax denominators) ->
            reciprocal + partition_broadcast + multiply -> C^T
  stage 3: out = C @ Wp^T + b  (C^T tiles feed matmul lhsT directly)
"""

import sys

sys.path.insert(0, "/opt/trn_rl_repo")

import ml_dtypes
import numpy as np

import concourse.bass as bass  # noqa: F401  (registers AP machinery)
import concourse.mybir as mybir
import concourse.tile as tile
from concourse import bacc, bass_utils

DIM = 768
H = 12
D = 64
N = 577
B = 32
NCORES = 8
BLOC = B // NCORES
SCALE = D**-0.5

BF16 = mybir.dt.bfloat16
F32 = mybir.dt.float32

# token/key chunks along a 577 axis mapped to <=128 partitions
PCH = [(0, 128), (128, 128), (256, 128), (384, 128), (512, 65)]
# free-dim chunks along a 577 axis (<=512 per PSUM bank)
NCH = [(0, 512), (512, 65)]
# free-dim chunks along the 768 output-feature axis
OCH = [(0, 512), (512, 256)]

_NC_CACHE = None


def _build(tc, xT, wqkT, wvT, wpT, bias, out):
    nc = tc.nc
    exp = mybir.ActivationFunctionType.Exp

    with (
        tc.tile_pool(name="w", bufs=1) as wpool,
        tc.tile_pool(name="xb", bufs=2) as xpool,
        tc.tile_pool(name="qk", bufs=2) as qkpool,
        tc.tile_pool(name="vp", bufs=2) as vppool,
        tc.tile_pool(name="es", bufs=6) as espool,
        tc.tile_pool(name="ct", bufs=2) as ctpool,
        tc.tile_pool(name="sm", bufs=6) as smpool,
        tc.tile_pool(name="ob", bufs=3) as opool,
        tc.tile_pool(name="ps", bufs=6, space="PSUM") as pspool,
        tc.tile_pool(name="pso", bufs=2, space="PSUM") as psopool,
    ):
        # Split the big weight loads per contraction-chunk so the first
        # matmuls can start as soon as chunk 0 lands.
        wqk_sb = wpool.tile([128, 6, 2 * DIM], BF16)
        wqk_dr = wqkT.ap().rearrange("(j p) o -> p j o", p=128)
        wv_sb = wpool.tile([128, 6, DIM], BF16)
        wv_dr = wvT.ap().rearrange("(j p) o -> p j o", p=128)
        wp_sb = wpool.tile([128, 6, DIM], BF16)
        wp_dr = wpT.ap().rearrange("(j p) o -> p j o", p=128)
        # Per-batch state, filled lazily by the emit helpers below.
        st = [dict() for _ in range(BLOC)]

        def load_x(b):
            xt = xpool.tile([128, 6, N], BF16, tag="xt")
            xt_dr = xT.ap()[b].rearrange("(j p) t -> p j t", p=128)
            for c in range(6):
                nc.sync.dma_start(xt[:, c], xt_dr[:, c])
            st[b]["xt"] = xt

        # First matmul needs xt(0) chunk 0 + wqk chunk 0 — issue those first;
        # wp/bias (stage-3 only) go last.
        xt0 = xpool.tile([128, 6, N], BF16, tag="xt")
        xt0_dr = xT.ap()[0].rearrange("(j p) t -> p j t", p=128)
        st[0]["xt"] = xt0
        for c in range(6):
            nc.sync.dma_start(xt0[:, c], xt0_dr[:, c])
            nc.sync.dma_start(wqk_sb[:, c], wqk_dr[:, c])
        for c in range(6):
            nc.sync.dma_start(wv_sb[:, c], wv_dr[:, c])
        for c in range(6):
            nc.sync.dma_start(wp_sb[:, c], wp_dr[:, c])
        bias_sb = wpool.tile([128, DIM], F32)
        nc.sync.dma_start(bias_sb[:], bias.ap())

        # Warm-up burst: the PE idles while the first DMAs land; dummy
        # matmuls there flip the HAM clock gate to 8/8 before real work.
        warm = wpool.tile([128, 512], BF16, name="warm")
        nc.gpsimd.memset(warm[:], 0.0)
        warm_ps = pspool.tile([128, 512], F32, tag="mm", name="warm_ps")
        for _ in range(35):
            nc.tensor.matmul(
                warm_ps[:, :], warm[:, 0:128], warm[:, 0:512], start=True, stop=True
            )

        # stage-1 work units: 12 QK o-tile groups (ordered so head-pair i's
        # Q and K tiles land early) + 10 V' groups = 22 units per batch.
        QK_ORDER = [0, 6, 1, 7, 2, 8, 3, 9, 4, 10, 5, 11]

        def stage1_unit(b, u):
            xt = st[b]["xt"]
            if u < 12:
                # QK^T [1536, 577]; o-tile j covers rows 128j..128j+127
                j = QK_ORDER[u]
                if "qk" not in st[b]:
                    st[b]["qk"] = qkpool.tile([128, 12, N], BF16, tag="qk", name="qk")
                qk = st[b]["qk"]
                for n0, nsz in NCH:
                    ps = pspool.tile([128, 512], F32, tag="mm")
                    for c in range(6):
                        nc.tensor.matmul(
                            ps[:, :nsz],
                            wqk_sb[:, c, j * 128 : (j + 1) * 128],
                            xt[:, c, n0 : n0 + nsz],
                            start=(c == 0),
                            stop=(c == 5),
                        )
                    nc.vector.tensor_copy(qk[:, j, n0 : n0 + nsz], ps[:, :nsz])
            else:
                # V' [577, 12*65]: per head 64 value cols + a ones col
                it, io = divmod(u - 12, 2)
                if "vp" not in st[b]:
                    vp = vppool.tile([128, 5, H * 65], BF16, tag="vp")
                    st[b]["vp"] = vp
                    for k in range(5):
                        ones = vp[:, k].rearrange("p (h c) -> p h c", c=65)[:, :, 64:65]
                        nc.gpsimd.memset(ones, 1.0)
                vp = st[b]["vp"]
                t0, tsz = PCH[it]
                o0, osz = OCH[io]
                ps = pspool.tile([128, 512], F32, tag="mm")
                for c in range(6):
                    nc.tensor.matmul(
                        ps[:tsz, :osz],
                        xt[:, c, t0 : t0 + tsz],
                        wv_sb[:, c, o0 : o0 + osz],
                        start=(c == 0),
                        stop=(c == 5),
                    )
                nh = osz // D
                h0 = o0 // D
                src = ps[:tsz, :osz].rearrange("p (h d) -> p h d", d=D)
                dst = vp[:tsz, it].rearrange("p (h c) -> p h c", c=65)[
                    :, h0 : h0 + nh, 0:D
                ]
                nc.vector.tensor_copy(dst, src)

        def scores_pair(b, i, fillers=None):
            # Heads 2i (partitions 0:64) and 2i+1 (64:128) sit in disjoint
            # PE row-groups; adjacent K=64 matmuls run concurrently in the
            # array (tile_position auto-derives from the partition bases).
            # Between m-chunk pairs, pop a filler thunk (stage-1/3 matmuls of
            # neighboring batches) so the PE has independent work while ACT
            # drains the exps that gate the next psum slot.
            qk = st[b]["qk"]
            esA = espool.tile([128, 5, N], BF16, tag="es")
            esB = espool.tile([128, 5, N], BF16, tag="es")
            st[b].setdefault("es", {})[2 * i] = esA
            st[b]["es"][2 * i + 1] = esB
            kA, qA = qk[0:64, 6 + i], qk[0:64, i]
            kB, qB = qk[64:128, 6 + i], qk[64:128, i]
            for mc, (m0, msz) in enumerate(PCH):
                psA = pspool.tile([128, 512], F32, tag="mm", name="psA")
                psB = pspool.tile([128, 512], F32, tag="mm", name="psB")
                nc.tensor.matmul(
                    psA[:msz, :512], kA[:, m0 : m0 + msz], qA[:, 0:512],
                    start=True, stop=True,
                )
                nc.tensor.matmul(
                    psB[:msz, :512], kB[:, m0 : m0 + msz], qB[:, 0:512],
                    start=True, stop=True,
                )
                nc.scalar.activation(esA[:msz, mc, 0:512], psA[:msz, :512], exp)
                nc.scalar.activation(esB[:msz, mc, 0:512], psB[:msz, :512], exp)
                if fillers:
                    fillers.popleft()()
            # the 65-wide query tail: all 5 key-chunks share one PSUM bank
            pstA_t = pspool.tile([128, 512], F32, tag="mm", name="pstA")
            pstB_t = pspool.tile([128, 512], F32, tag="mm", name="pstB")
            pstA = pstA_t[:, 0:325].rearrange("p (m c) -> p m c", c=65)
            pstB = pstB_t[:, 0:325].rearrange("p (m c) -> p m c", c=65)
            for mc, (m0, msz) in enumerate(PCH):
                nc.tensor.matmul(
                    pstA[:msz, mc, :], kA[:, m0 : m0 + msz], qA[:, 512:577],
                    start=True, stop=True,
                )
                nc.tensor.matmul(
                    pstB[:msz, mc, :], kB[:, m0 : m0 + msz], qB[:, 512:577],
                    start=True, stop=True,
                )
            nc.scalar.activation(esA[:, :, 512:577], pstA[:, :, :], exp)
            nc.scalar.activation(esB[:, :, 512:577], pstB[:, :, :], exp)

        def pv_norm(b, h):
            jq = h // 2
            pq = (h % 2) * 64
            vp = st[b]["vp"]
            es = st[b]["es"].pop(h)
            if "ct" not in st[b]:
                st[b]["ct"] = ctpool.tile([128, 6, N], BF16, tag="ct", name="ct")
            ct = st[b]["ct"]
            # O'^T = V'.T @ expS^T ; row 64 = per-query softmax denominator.
            # The two query chunks have independent denominators, so each
            # chunk's normalize chain starts as soon as its PV group lands.
            for n0, nsz in NCH:
                pso = psopool.tile([65, 512], F32, tag="pv")
                for mc, (m0, msz) in enumerate(PCH):
                    nc.tensor.matmul(
                        pso[:65, :nsz],
                        vp[:msz, mc, h * 65 : (h + 1) * 65],
                        es[:msz, mc, n0 : n0 + nsz],
                        start=(mc == 0),
                        stop=(mc == 4),
                    )
                # custom-DVE recip mis-reads PSUM; bounce the sums row to SBUF
                sums = smpool.tile([1, 512], F32, tag="sums")
                nc.vector.tensor_copy(sums[0:1, :nsz], pso[64:65, :nsz])
                recip = smpool.tile([1, 512], F32, tag="recip")
                nc.vector.reciprocal_approx_fast(recip[0:1, :nsz], sums[0:1, :nsz])
                rb = smpool.tile([64, 512], F32, tag="rb")
                nc.gpsimd.partition_broadcast(rb[0:64, :nsz], recip[0:1, :nsz])
                nc.vector.tensor_mul(
                    ct[pq : pq + 64, jq, n0 : n0 + nsz],
                    pso[0:64, :nsz],
                    rb[0:64, :nsz],
                )

        def stage3_half(b, it, io):
            # out = C @ Wp^T + b for one (token chunk, feature chunk)
            ct = st[b]["ct"]
            t0, tsz = PCH[it]
            o0, osz = OCH[io]
            if io == 0:
                st[b].setdefault("ob", {})[it] = opool.tile(
                    [128, DIM], F32, tag="ob", name="ob"
                )
            ob = st[b]["ob"][it]
            ps = pspool.tile([128, 512], F32, tag="mm")
            for c in range(6):
                nc.tensor.matmul(
                    ps[:tsz, :osz],
                    ct[:, c, t0 : t0 + tsz],
                    wp_sb[:, c, o0 : o0 + osz],
                    start=(c == 0),
                    stop=(c == 5),
                )
            nc.vector.tensor_add(
                ob[:tsz, o0 : o0 + osz],
                ps[:tsz, :osz],
                bias_sb[:tsz, o0 : o0 + osz],
            )
            if io == 1:
                nc.sync.dma_start(out.ap()[b, t0 : t0 + tsz, :], ob[:tsz, :])

        def stage3_unit(b, it):
            stage3_half(b, it, 0)
            stage3_half(b, it, 1)

        # ---- cross-batch software pipeline.  During stage-2 of batch b
        # (where the PE repeatedly waits on ACT exps), interleave the
        # dependency-free stage-1 matmuls of batch b+1 and the stage-3
        # matmuls of batch b-1 as PE filler work.
        from collections import deque

        # Stage-1 of batch b splits into a prologue (first head-pair's QK
        # tiles + all of V') emitted during stage-2 of b-1, and "self" QK
        # units emitted inside b's own stage-2 one pair ahead of use — so
        # even the last batch's stage-2 has PE filler work.
        PRO = [0, 1] + list(range(12, 22))
        SELF = list(range(2, 12))

        for u in PRO:
            stage1_unit(0, u)
        load_x(1)
        for b in range(BLOC):
            last = b == BLOC - 1
            npairs = H // 2 - 1 if last else H // 2
            for i in range(npairs):
                fillers = deque()
                if i < 5:
                    for u in (SELF[2 * i], SELF[2 * i + 1]):
                        fillers.append(lambda b=b, u=u: stage1_unit(b, u))
                if b + 1 < BLOC:
                    for u in PRO[2 * i : 2 * i + 2]:
                        fillers.append(lambda b=b, u=u: stage1_unit(b + 1, u))
                if b >= 1 and i >= 1:
                    for k in (2 * (i - 1), 2 * (i - 1) + 1):
                        it, io = divmod(k, 2)
                        fillers.append(
                            lambda b=b, it=it, io=io: stage3_half(b - 1, it, io)
                        )
                scores_pair(b, i, fillers)
                if i >= 1:
                    pv_norm(b, 2 * i - 2)
                while fillers:
                    fillers.popleft()()
                if i >= 1:
                    pv_norm(b, 2 * i - 1)
            if last:
                # emit the final pair's scores early so ACT's last exps
                # start a slot sooner; its stage-3 fillers ride along
                fillers = deque()
                for k in (8, 9):
                    it, io = divmod(k, 2)
                    fillers.append(
                        lambda b=b, it=it, io=io: stage3_half(b - 1, it, io)
                    )
                scores_pair(b, 5, fillers)
                pv_norm(b, 8)
                while fillers:
                    fillers.popleft()()
                pv_norm(b, 9)
            if b + 2 < BLOC:
                load_x(b + 2)
            pv_norm(b, H - 2)
            pv_norm(b, H - 1)
        for it in range(5):
            stage3_unit(BLOC - 1, it)


def _build_nc():
    global _NC_CACHE
    if _NC_CACHE is not None:
        return _NC_CACHE
    nc = bacc.Bacc("TRN2", target_bir_lowering=False, debug=False)
    xT = nc.dram_tensor("xT", [BLOC, DIM, N], BF16, kind="ExternalInput")
    wqkT = nc.dram_tensor("wqkT", [DIM, 2 * DIM], BF16, kind="ExternalInput")
    wvT = nc.dram_tensor("wvT", [DIM, DIM], BF16, kind="ExternalInput")
    wpT = nc.dram_tensor("wpT", [DIM, DIM], BF16, kind="ExternalInput")
    bias = nc.dram_tensor("bias", [128, DIM], F32, kind="ExternalInput")
    out = nc.dram_tensor("out", [BLOC, N, DIM], F32, kind="ExternalOutput")
    with tile.TileContext(nc) as tc:
        _build(tc, xT, wqkT, wvT, wpT, bias, out)
    nc.compile()
    _NC_CACHE = nc
    return nc


def _prep_inputs(x, W_qkv, W_proj, b_proj):
    bf = ml_dtypes.bfloat16
    x = np.asarray(x, dtype=np.float32)
    W_qkv = np.asarray(W_qkv, dtype=np.float32)
    W_proj = np.asarray(W_proj, dtype=np.float32)
    b_proj = np.asarray(b_proj, dtype=np.float32)

    wq = W_qkv[:DIM] * np.float32(SCALE)
    wk = W_qkv[DIM : 2 * DIM]
    wv = W_qkv[2 * DIM :]
    wqkT = np.ascontiguousarray(np.concatenate([wq, wk], axis=0).T).astype(bf)
    wvT = np.ascontiguousarray(wv.T).astype(bf)
    wpT = np.ascontiguousarray(W_proj.T).astype(bf)
    bias_bc = np.ascontiguousarray(np.broadcast_to(b_proj, (128, DIM))).astype(
        np.float32
    )

    in_maps = []
    for c in range(NCORES):
        xb = x[c * BLOC : (c + 1) * BLOC]  # [BLOC, N, DIM]
        xTc = np.ascontiguousarray(xb.transpose(0, 2, 1)).astype(bf)
        in_maps.append(
            {"xT": xTc, "wqkT": wqkT, "wvT": wvT, "wpT": wpT, "bias": bias_bc}
        )
    return in_maps


def _run(x, W_qkv, W_proj, b_proj, trace=False):
    nc = _build_nc()
    in_maps = _prep_inputs(x, W_qkv, W_proj, b_proj)
    res = bass_utils.run_bass_kernel_spmd(
        nc, in_maps, core_ids=list(range(NCORES)), trace=trace
    )
    out = np.concatenate(
        [np.asarray(res.results[c]["out"], dtype=np.float32) for c in range(NCORES)],
        axis=0,
    )
    return out, res


def kernel(x, W_qkv, W_proj, b_proj):
    out, _ = _run(x, W_qkv, W_proj, b_proj, trace=False)
    return out


# revision 57
# speedup vs baseline: 1.0174x; 1.0006x over previous
"""Multi-head attention (ViT-style, B=32 N=577 C=768 H=12) on 8 TRN2 NeuronCores.

Sharding: pure data-parallel over batch — each core gets 4 batches plus a
replicated copy of the (host-preprocessed) weights. No collectives.

Per-core pipeline (all matmuls bf16 with fp32 PSUM accumulation):
  stage 1a: QK^T = [Wq*scale; Wk]^T-major matmul  -> qk  [1536, 577] (o-major)
  stage 1b: V'   = x @ Wv^T laid out per-head with a ones column  (for softmax sums)
  stage 2 per head: S^T = K^T.T@Q^T -> exp (no max-sub; scores are O(1)) ->
            O'^T = V'.T @ expS^T  (row 64 = softmax denominators) ->
            reciprocal + partition_broadcast + multiply -> C^T
  stage 3: out = C @ Wp^T + b  (C^T tiles feed matmul lhsT directly)
"""

import sys

sys.path.insert(0, "/opt/trn_rl_repo")

import ml_dtypes
import numpy as np

import concourse.bass as bass  # noqa: F401  (registers AP machinery)
import concourse.mybir as mybir
import concourse.tile as tile
from concourse import bacc, bass_utils

DIM = 768
H = 12
D = 64
N = 577
B = 32
NCORES = 8
BLOC = B // NCORES
SCALE = D**-0.5

BF16 = mybir.dt.bfloat16
F32 = mybir.dt.float32

# token/key chunks along a 577 axis mapped to <=128 partitions
PCH = [(0, 128), (128, 128), (256, 128), (384, 128), (512, 65)]
# free-dim chunks along a 577 axis (<=512 per PSUM bank)
NCH = [(0, 512), (512, 65)]
# free-dim chunks along the 768 output-feature axis
OCH = [(0, 512), (512, 256)]

_NC_CACHE = None


def _build(tc, xT, wqkT, wvT, wpT, bias, out):
    nc = tc.nc
    exp = mybir.ActivationFunctionType.Exp

    with (
        tc.tile_pool(name="w", bufs=1) as wpool,
        tc.tile_pool(name="xb", bufs=2) as xpool,
        tc.tile_pool(name="qk", bufs=2) as qkpool,
        tc.tile_pool(name="vp", bufs=2) as vppool,
        tc.tile_pool(name="es", bufs=6) as espool,
        tc.tile_pool(name="ct", bufs=2) as ctpool,
        tc.tile_pool(name="sm", bufs=6) as smpool,
        tc.tile_pool(name="ob", bufs=3) as opool,
        tc.tile_pool(name="ps", bufs=6, space="PSUM") as pspool,
        tc.tile_pool(name="pso", bufs=2, space="PSUM") as psopool,
    ):
        # Split the big weight loads per contraction-chunk so the first
        # matmuls can start as soon as chunk 0 lands.
        wqk_sb = wpool.tile([128, 6, 2 * DIM], BF16)
        wqk_dr = wqkT.ap().rearrange("(j p) o -> p j o", p=128)
        wv_sb = wpool.tile([128, 6, DIM], BF16)
        wv_dr = wvT.ap().rearrange("(j p) o -> p j o", p=128)
        wp_sb = wpool.tile([128, 6, DIM], BF16)
        wp_dr = wpT.ap().rearrange("(j p) o -> p j o", p=128)
        # Per-batch state, filled lazily by the emit helpers below.
        st = [dict() for _ in range(BLOC)]

        def load_x(b):
            xt = xpool.tile([128, 6, N], BF16, tag="xt")
            xt_dr = xT.ap()[b].rearrange("(j p) t -> p j t", p=128)
            for c in range(6):
                nc.sync.dma_start(xt[:, c], xt_dr[:, c])
            st[b]["xt"] = xt

        # First matmul needs xt(0) chunk 0 + wqk chunk 0 — issue those first;
        # wp/bias (stage-3 only) go last.
        xt0 = xpool.tile([128, 6, N], BF16, tag="xt")
        xt0_dr = xT.ap()[0].rearrange("(j p) t -> p j t", p=128)
        st[0]["xt"] = xt0
        for c in range(6):
            nc.sync.dma_start(xt0[:, c], xt0_dr[:, c])
            nc.sync.dma_start(wqk_sb[:, c], wqk_dr[:, c])
        for c in range(6):
            nc.sync.dma_start(wv_sb[:, c], wv_dr[:, c])
        for c in range(6):
            nc.sync.dma_start(wp_sb[:, c], wp_dr[:, c])
        bias_sb = wpool.tile([128, DIM], F32)
        nc.sync.dma_start(bias_sb[:], bias.ap())

        # Warm-up burst: the PE idles while the first DMAs land; dummy
        # matmuls there flip the HAM clock gate to 8/8 before real work.
        warm = wpool.tile([128, 512], BF16, name="warm")
        nc.gpsimd.memset(warm[:], 0.0)
        warm_ps = pspool.tile([128, 512], F32, tag="mm", name="warm_ps")
        for _ in range(35):
            nc.tensor.matmul(
                warm_ps[:, :], warm[:, 0:128], warm[:, 0:512], start=True, stop=True
            )

        # stage-1 work units: 12 QK o-tile groups (ordered so head-pair i's
        # Q and K tiles land early) + 10 V' groups = 22 units per batch.
        QK_ORDER = [0, 6, 1, 7, 2, 8, 3, 9, 4, 10, 5, 11]

        def stage1_unit(b, u):
            xt = st[b]["xt"]
            if u < 12:
                # QK^T [1536, 577]; o-tile j covers rows 128j..128j+127
                j = QK_ORDER[u]
                if "qk" not in st[b]:
                    st[b]["qk"] = qkpool.tile([128, 12, N], BF16, tag="qk", name="qk")
                qk = st[b]["qk"]
                for n0, nsz in NCH:
                    ps = pspool.tile([128, 512], F32, tag="mm")
                    for c in range(6):
                        nc.tensor.matmul(
                            ps[:, :nsz],
                            wqk_sb[:, c, j * 128 : (j + 1) * 128],
                            xt[:, c, n0 : n0 + nsz],
                            start=(c == 0),
                            stop=(c == 5),
                        )
                    nc.vector.tensor_copy(qk[:, j, n0 : n0 + nsz], ps[:, :nsz])
            else:
                # V' [577, 12*65]: per head 64 value cols + a ones col
                it, io = divmod(u - 12, 2)
                if "vp" not in st[b]:
                    vp = vppool.tile([128, 5, H * 65], BF16, tag="vp")
                    st[b]["vp"] = vp
                    for k in range(5):
                        ones = vp[:, k].rearrange("p (h c) -> p h c", c=65)[:, :, 64:65]
                        nc.gpsimd.memset(ones, 1.0)
                vp = st[b]["vp"]
                t0, tsz = PCH[it]
                o0, osz = OCH[io]
                ps = pspool.tile([128, 512], F32, tag="mm")
                for c in range(6):
                    nc.tensor.matmul(
                        ps[:tsz, :osz],
                        xt[:, c, t0 : t0 + tsz],
                        wv_sb[:, c, o0 : o0 + osz],
                        start=(c == 0),
                        stop=(c == 5),
                    )
                nh = osz // D
                h0 = o0 // D
                src = ps[:tsz, :osz].rearrange("p (h d) -> p h d", d=D)
                dst = vp[:tsz, it].rearrange("p (h c) -> p h c", c=65)[
                    :, h0 : h0 + nh, 0:D
                ]
                nc.vector.tensor_copy(dst, src)

        def scores_pair(b, i, fillers=None):
            # Heads 2i (partitions 0:64) and 2i+1 (64:128) sit in disjoint
            # PE row-groups; adjacent K=64 matmuls run concurrently in the
            # array (tile_position auto-derives from the partition bases).
            # Between m-chunk pairs, pop a filler thunk (stage-1/3 matmuls of
            # neighboring batches) so the PE has independent work while ACT
            # drains the exps that gate the next psum slot.
            qk = st[b]["qk"]
            esA = espool.tile([128, 5, N], BF16, tag="es")
            esB = espool.tile([128, 5, N], BF16, tag="es")
            st[b].setdefault("es", {})[2 * i] = esA
            st[b]["es"][2 * i + 1] = esB
            kA, qA = qk[0:64, 6 + i], qk[0:64, i]
            kB, qB = qk[64:128, 6 + i], qk[64:128, i]
            for mc, (m0, msz) in enumerate(PCH):
                psA = pspool.tile([128, 512], F32, tag="mm", name="psA")
                psB = pspool.tile([128, 512], F32, tag="mm", name="psB")
                nc.tensor.matmul(
                    psA[:msz, :512], kA[:, m0 : m0 + msz], qA[:, 0:512],
                    start=True, stop=True,
                )
                nc.tensor.matmul(
                    psB[:msz, :512], kB[:, m0 : m0 + msz], qB[:, 0:512],
                    start=True, stop=True,
                )
                nc.scalar.activation(esA[:msz, mc, 0:512], psA[:msz, :512], exp)
                nc.scalar.activation(esB[:msz, mc, 0:512], psB[:msz, :512], exp)
                if fillers:
                    fillers.popleft()()
            # the 65-wide query tail: all 5 key-chunks share one PSUM bank
            pstA_t = pspool.tile([128, 512], F32, tag="mm", name="pstA")
            pstB_t = pspool.tile([128, 512], F32, tag="mm", name="pstB")
            pstA = pstA_t[:, 0:325].rearrange("p (m c) -> p m c", c=65)
            pstB = pstB_t[:, 0:325].rearrange("p (m c) -> p m c", c=65)
            for mc, (m0, msz) in enumerate(PCH):
                nc.tensor.matmul(
                    pstA[:msz, mc, :], kA[:, m0 : m0 + msz], qA[:, 512:577],
                    start=True, stop=True,
                )
                nc.tensor.matmul(
                    pstB[:msz, mc, :], kB[:, m0 : m0 + msz], qB[:, 512:577],
                    start=True, stop=True,
                )
            nc.scalar.activation(esA[:, :, 512:577], pstA[:, :, :], exp)
            nc.scalar.activation(esB[:, :, 512:577], pstB[:, :, :], exp)

        def pv_norm(b, h):
            jq = h // 2
            pq = (h % 2) * 64
            vp = st[b]["vp"]
            es = st[b]["es"].pop(h)
            if "ct" not in st[b]:
                st[b]["ct"] = ctpool.tile([128, 6, N], BF16, tag="ct", name="ct")
            ct = st[b]["ct"]
            # O'^T = V'.T @ expS^T ; row 64 = per-query softmax denominator.
            # The two query chunks have independent denominators, so each
            # chunk's normalize chain starts as soon as its PV group lands.
            for n0, nsz in NCH:
                pso = psopool.tile([65, 512], F32, tag="pv")
                for mc, (m0, msz) in enumerate(PCH):
                    nc.tensor.matmul(
                        pso[:65, :nsz],
                        vp[:msz, mc, h * 65 : (h + 1) * 65],
                        es[:msz, mc, n0 : n0 + nsz],
                        start=(mc == 0),
                        stop=(mc == 4),
                    )
                # custom-DVE recip mis-reads PSUM; bounce the sums row to SBUF
                sums = smpool.tile([1, 512], F32, tag="sums")
                nc.vector.tensor_copy(sums[0:1, :nsz], pso[64:65, :nsz])
                recip = smpool.tile([1, 512], F32, tag="recip")
                nc.vector.reciprocal_approx_fast(recip[0:1, :nsz], sums[0:1, :nsz])
                rb = smpool.tile([64, 512], F32, tag="rb")
                nc.gpsimd.partition_broadcast(rb[0:64, :nsz], recip[0:1, :nsz])
                nc.vector.tensor_mul(
                    ct[pq : pq + 64, jq, n0 : n0 + nsz],
                    pso[0:64, :nsz],
                    rb[0:64, :nsz],
                )

        def stage3_half(b, it, io):
            # out = C @ Wp^T + b for one (token chunk, feature chunk)
            ct = st[b]["ct"]
            t0, tsz = PCH[it]
            o0, osz = OCH[io]
            if io == 0:
                st[b].setdefault("ob", {})[it] = opool.tile(
                    [128, DIM], F32, tag="ob", name="ob"
                )
            ob = st[b]["ob"][it]
            ps = pspool.tile([128, 512], F32, tag="mm")
            for c in range(6):
                nc.tensor.matmul(
                    ps[:tsz, :osz],
                    ct[:, c, t0 : t0 + tsz],
                    wp_sb[:, c, o0 : o0 + osz],
                    start=(c == 0),
                    stop=(c == 5),
                )
            nc.vector.tensor_add(
                ob[:tsz, o0 : o0 + osz],
                ps[:tsz, :osz],
                bias_sb[:tsz, o0 : o0 + osz],
            )
            if io == 1:
                nc.sync.dma_start(out.ap()[b, t0 : t0 + tsz, :], ob[:tsz, :])

        def stage3_unit(b, it):
            stage3_half(b, it, 0)
            stage3_half(b, it, 1)

        # ---- cross-batch software pipeline.  During stage-2 of batch b
        # (where the PE repeatedly waits on ACT exps), interleave the
        # dependency-free stage-1 matmuls of batch b+1 and the stage-3
        # matmuls of batch b-1 as PE filler work.
        from collections import deque

        # Stage-1 of batch b splits into a prologue (first head-pair's QK
        # tiles + all of V') emitted during stage-2 of b-1, and "self" QK
        # units emitted inside b's own stage-2 one pair ahead of use — so
        # even the last batch's stage-2 has PE filler work.
        PRO = [0, 1] + list(range(12, 22))
        SELF = list(range(2, 12))

        for u in PRO:
            stage1_unit(0, u)
        load_x(1)
        for b in range(BLOC):
            last = b == BLOC - 1
            npairs = H // 2 - 1 if last else H // 2
            for i in range(npairs):
                fillers = deque()
                if i < 5:
                    for u in (SELF[2 * i], SELF[2 * i + 1]):
                        fillers.append(lambda b=b, u=u: stage1_unit(b, u))
                if b + 1 < BLOC:
                    for u in PRO[2 * i : 2 * i + 2]:
                        fillers.append(lambda b=b, u=u: stage1_unit(b + 1, u))
                if b >= 1 and i >= 1:
                    ks = [i - 1] if last else [2 * (i - 1), 2 * (i - 1) + 1]
                    for k in ks:
                        it, io = divmod(k, 2)
                        fillers.append(
                            lambda b=b, it=it, io=io: stage3_half(b - 1, it, io)
                        )
                scores_pair(b, i, fillers)
                if i >= 1:
                    pv_norm(b, 2 * i - 2)
                while fillers:
                    fillers.popleft()()
                if i >= 1:
                    pv_norm(b, 2 * i - 1)
            if last:
                # emit the final pair's scores early so ACT's last exps
                # start a slot sooner; remaining stage-3 halves of b-1
                # interleave between the trailing pv chains to cover their
                # DVE normalize latency
                fillers = deque()
                for k in (4, 5):
                    it, io = divmod(k, 2)
                    fillers.append(
                        lambda b=b, it=it, io=io: stage3_half(b - 1, it, io)
                    )
                scores_pair(b, 5, fillers)
                pv_norm(b, 8)
                while fillers:
                    fillers.popleft()()
                stage3_half(b - 1, 3, 0)
                pv_norm(b, 9)
                stage3_half(b - 1, 3, 1)
                pv_norm(b, 10)
                stage3_half(b - 1, 4, 0)
                pv_norm(b, 11)
                stage3_half(b - 1, 4, 1)
            else:
                if b + 2 < BLOC:
                    load_x(b + 2)
                pv_norm(b, H - 2)
                pv_norm(b, H - 1)
        for it in range(5):
            stage3_unit(BLOC - 1, it)


def _build_nc():
    global _NC_CACHE
    if _NC_CACHE is not None:
        return _NC_CACHE
    nc = bacc.Bacc("TRN2", target_bir_lowering=False, debug=False)
    xT = nc.dram_tensor("xT", [BLOC, DIM, N], BF16, kind="ExternalInput")
    wqkT = nc.dram_tensor("wqkT", [DIM, 2 * DIM], BF16, kind="ExternalInput")
    wvT = nc.dram_tensor("wvT", [DIM, DIM], BF16, kind="ExternalInput")
    wpT = nc.dram_tensor("wpT", [DIM, DIM], BF16, kind="ExternalInput")
    bias = nc.dram_tensor("bias", [128, DIM], F32, kind="ExternalInput")
    out = nc.dram_tensor("out", [BLOC, N, DIM], F32, kind="ExternalOutput")
    with tile.TileContext(nc) as tc:
        _build(tc, xT, wqkT, wvT, wpT, bias, out)
    nc.compile()
    _NC_CACHE = nc
    return nc


def _prep_inputs(x, W_qkv, W_proj, b_proj):
    bf = ml_dtypes.bfloat16
    x = np.asarray(x, dtype=np.float32)
    W_qkv = np.asarray(W_qkv, dtype=np.float32)
    W_proj = np.asarray(W_proj, dtype=np.float32)
    b_proj = np.asarray(b_proj, dtype=np.float32)

    wq = W_qkv[:DIM] * np.float32(SCALE)
    wk = W_qkv[DIM : 2 * DIM]
    wv = W_qkv[2 * DIM :]
    wqkT = np.ascontiguousarray(np.concatenate([wq, wk], axis=0).T).astype(bf)
    wvT = np.ascontiguousarray(wv.T).astype(bf)
    wpT = np.ascontiguousarray(W_proj.T).astype(bf)
    bias_bc = np.ascontiguousarray(np.broadcast_to(b_proj, (128, DIM))).astype(
        np.float32
    )

    in_maps = []
    for c in range(NCORES):
        xb = x[c * BLOC : (c + 1) * BLOC]  # [BLOC, N, DIM]
        xTc = np.ascontiguousarray(xb.transpose(0, 2, 1)).astype(bf)
        in_maps.append(
            {"xT": xTc, "wqkT": wqkT, "wvT": wvT, "wpT": wpT, "bias": bias_bc}
        )
    return in_maps


def _run(x, W_qkv, W_proj, b_proj, trace=False):
    nc = _build_nc()
    in_maps = _prep_inputs(x, W_qkv, W_proj, b_proj)
    res = bass_utils.run_bass_kernel_spmd(
        nc, in_maps, core_ids=list(range(NCORES)), trace=trace
    )
    out = np.concatenate(
        [np.asarray(res.results[c]["out"], dtype=np.float32) for c in range(NCORES)],
        axis=0,
    )
    return out, res


def kernel(x, W_qkv, W_proj, b_proj):
    out, _ = _run(x, W_qkv, W_proj, b_proj, trace=False)
    return out
